# revision 1
# baseline (speedup 1.0000x reference)
"""XNOR-Net BasicBlock (BN-sign-conv x2 + residual, training-mode BN) on 8 TRN2 cores.

Strategy (data-parallel on batch, 4 images/core):
  phase0: per-channel sum/sumsq of x        -> AllReduce (BN1 stats)
  conv1 : s1 = sign(x - t1) (fp8 +-1), 3x3 conv as 9 DoubleRow fp8 matmuls per
          output tile (integer-exact), epilogue u1 = prelu(y1) kept fp16 in
          SBUF + per-channel sum/sumsq of u1 -> AllReduce (BN2 stats)
  conv2 : s2 = sign(u1 - theta2), same conv, u2 = prelu-units kept fp16,
          stats -> AllReduce (BN3 stats)
  phase3: out = prelu(K*u2 + D + x, a3)  (BN3 affine folded into K, D)

Weights are binarized/laid out on host (tiny); all heavy tensors flow on device.
"""

import sys

sys.path.insert(0, "/opt/trn_rl_repo")

import numpy as np

import concourse.bacc as bacc
import concourse.mybir as mybir
import concourse.tile as tile
from concourse.bass_utils import run_bass_kernel_spmd

F32 = mybir.dt.float32
F16 = mybir.dt.float16
F8 = mybir.dt.float8e4
F8NP = mybir.dt.np(F8)

AF = mybir.ActivationFunctionType
OP = mybir.AluOpType
DR = mybir.MatmulPerfMode.DoubleRow

NCORES = 8
B, C, H, W = 32, 256, 56, 56
BL = B // NCORES          # images per core
HW = H * W                # 3136
PW = W + 2                # 58 padded width
PLANE = PW * PW           # 3364 padded plane (58 rows x 58 cols)
PLANE_PAD = 3392          # plane stride, %16 == 0
BAND = 8                  # output rows per matmul tile
NBAND = H // BAND         # 7
NFREE = BAND * PW         # 464 psum free size
NTOT = B * HW             # BN count (N*H*W over full batch)
EPS = 1e-5
OFFS = [(dh, dw) for dh in range(3) for dw in range(3)]

# cvec column indices
CV_B1, CV_C1M, CV_SF1N, CV_SF1SQN, CV_B2, CV_ISF1, CV_IA1, CV_C2M, \
    CV_SF2N, CV_SF2SQN, CV_B3, CV_G3SF2, CV_G3, CV_B3V, CV_A3, CV_EPS = range(16)
CV_NCOLS = 16

_CACHE = {}


def _build():
    nc = bacc.Bacc(num_devices=NCORES)
    x_d = nc.declare_dram_parameter("x", [BL, C, H, W], F32, isOutput=False)
    w1_d = nc.declare_dram_parameter("w1s", [128, 2, 18 * 128], F8, isOutput=False)
    w2_d = nc.declare_dram_parameter("w2s", [128, 2, 18 * 128], F8, isOutput=False)
    cv_d = nc.declare_dram_parameter("cvec", [128, 2, CV_NCOLS], F32, isOutput=False)
    out_d = nc.declare_dram_parameter("out", [BL, C, H, W], F32, isOutput=True)

    # DRAM-side views: channel c -> (g = c // 128, p = c % 128)
    def x_view(n):
        return x_d[n].rearrange("(g p) h w -> p g (h w)", p=128)

    def out_view(n, g):
        return out_d[n].rearrange("(g p) h w -> p g h w", p=128)[:, g]

    with tile.TileContext(nc, num_cores=NCORES, pool_alloc_mode="queue") as tc:
        import contextlib

        es_u1 = contextlib.ExitStack()
        es_u2 = contextlib.ExitStack()
        es_xs = contextlib.ExitStack()
        with tc.tile_pool(name="consts", bufs=1) as cpool, \
                tc.tile_pool(name="weights", bufs=1) as wpool, \
                tc.tile_pool(name="spool", bufs=1) as spool, \
                tc.tile_pool(name="scr", bufs=3) as scrpool, \
                tc.tile_pool(name="psum", bufs=8, space="PSUM") as psum_pool, \
                tc.tile_pool(name="dram", bufs=1, space="DRAM") as dram_pool, \
                es_u2:

            # ---- persistent small tiles ----
            cvec = cpool.tile([128, 2, CV_NCOLS], F32, tag="cvec")
            st1 = cpool.tile([128, 16], F32, tag="st1")
            st2 = cpool.tile([128, 112], F32, tag="st2")
            st3 = cpool.tile([128, 112], F32, tag="st3")
            g1 = cpool.tile([128, 4], F32, tag="g1")
            g2 = cpool.tile([128, 4], F32, tag="g2")
            g3t = cpool.tile([128, 4], F32, tag="g3t")
            negt1 = cpool.tile([128, 2], F32, tag="negt1")
            negth2 = cpool.tile([128, 2], F32, tag="negth2")
            kvec = cpool.tile([128, 2], F32, tag="kvec")
            dvec = cpool.tile([128, 2], F32, tag="dvec")
            tmp_a = cpool.tile([128, 2], F32, tag="tmp_a")
            tmp_b = cpool.tile([128, 2], F32, tag="tmp_b")
            tmp_c = cpool.tile([128, 2], F32, tag="tmp_c")

            w1t = wpool.tile([128, 2, 18 * 128], F8, tag="w1t")
            w2t = wpool.tile([128, 2, 18 * 128], F8, tag="w2t")
            z16 = cpool.tile([128, 2 * BAND, W], F16, tag="z16")

            # pool open order fixes the release order: u2 outlives u1
            # outlives xs; ph3 pools open after u1's region is freed
            u2_pool = es_u2.enter_context(tc.tile_pool(name="u2", bufs=BL))
            u1_pool = es_u1.enter_context(tc.tile_pool(name="u1", bufs=BL))
            xpool = es_xs.enter_context(tc.tile_pool(name="xs", bufs=3))

            s_tiles = [
                spool.tile([128, 2, PLANE_PAD], F8, tag="sa", name="sa"),
                spool.tile([128, 2, PLANE_PAD], F8, tag="sb", name="sb"),
            ]

            nc.sync.dma_start(cvec[:], cv_d[:])
            nc.sync.dma_start(w1t[:], w1_d[:])
            nc.sync.dma_start(w2t[:], w2_d[:])
            for s in s_tiles:
                nc.gpsimd.memset(s[:], 0.0)
            nc.gpsimd.memset(z16[:], 0.0)

            def s_plane(s, g):
                return s[:, g, 0:PLANE].rearrange("p (r w) -> p r w", w=PW)

            # =============== phase 0: x stats ===============
            # st1 col j = (k*2 + g)*4 + n   (k: 0=sum, 1=sumsq)
            for n in range(BL):
                xt = xpool.tile([128, 2, HW], F32, tag="xt")
                for g in range(2):
                    # half-image loads so stats start before the full image lands
                    nc.sync.dma_start(xt[:, g, :], x_view(n)[:, g, :])
                    nc.vector.reduce_sum(
                        st1[:, (0 * 2 + g) * 4 + n:(0 * 2 + g) * 4 + n + 1],
                        xt[:, g, :], axis=mybir.AxisListType.X,
                    )
                    # in-place square on ACT; tile is dead after this
                    nc.scalar.activation(
                        xt[:, g, :], xt[:, g, :], AF.Square, bias=0.0, scale=1.0,
                        accum_out=st1[:, (1 * 2 + g) * 4 + n:(1 * 2 + g) * 4 + n + 1],
                    )

            # reduce st1 [128, (k g) 4] -> g-pre [128, 4], AllReduce -> g1
            r1 = cpool.tile([128, 4], F32, tag="r1")
            nc.vector.reduce_sum(
                r1[:].rearrange("p (a b) -> p a b", b=1),
                st1[:].rearrange("p (kg t) -> p kg t", t=BL),
                axis=mybir.AxisListType.X,
            )
            ar1_i = dram_pool.tile([128, 4], F32, tag="ar1_i")
            ar1_o = dram_pool.tile([NCORES, 128, 4], F32, tag="ar1_o", addr_space="Shared")
            nc.sync.dma_start(ar1_i[:], r1[:])
            nc.gpsimd.collective_compute(
                "AllGather", OP.bypass, replica_groups=[list(range(NCORES))],
                ins=[ar1_i[:].opt()], outs=[ar1_o[:].opt()],
            )
            # g1 read on the ACT hwdge queue so SP can run the x prefetches
            # below during the collective
            gth1 = cpool.tile([128, 4, NCORES], F32, tag="gth1")
            nc.scalar.dma_start(gth1[:], ar1_o[:].rearrange("r p k -> p k r"))
            nc.vector.reduce_sum(
                g1[:].rearrange("p (a b) -> p a b", b=1), gth1[:],
                axis=mybir.AxisListType.X,
            )

            # prefetch conv1 prep x tiles; transfers overlap AR1
            x1s = []
            for n in range(BL):
                xt = xpool.tile([128, 2, HW], F32, tag="xt", name=f"x1_{n}")
                nc.sync.dma_start(xt[:], x_view(n))
                x1s.append(xt)

            # ---- BN1 threshold: negt1 = B1*std1 - m1 ----
            g1v = g1[:].rearrange("p (k g) -> p k g", k=2)
            nc.vector.tensor_scalar_mul(tmp_a[:], g1v[:, 0], 1.0 / NTOT)      # m1
            nc.vector.tensor_scalar_mul(tmp_b[:], g1v[:, 1], 1.0 / NTOT)      # E[x^2]
            nc.vector.scalar_tensor_tensor(                                   # v1 = E - m^2... (m*-1*m)+E
                tmp_c[:], tmp_a[:], -1.0, tmp_a[:], op0=OP.mult, op1=OP.mult,
            )
            nc.vector.tensor_add(tmp_c[:], tmp_c[:], tmp_b[:])                # v1
            nc.scalar.activation(tmp_b[:], tmp_c[:], AF.Sqrt, bias=cvec[:, 0, CV_EPS:CV_EPS + 1], scale=1.0)  # std1
            nc.vector.tensor_mul(tmp_c[:], tmp_b[:], cvec[:, :, CV_B1])       # B1*std1
            nc.vector.tensor_sub(negt1[:], tmp_c[:], tmp_a[:])                # B1*std1 - m1

            # =============== conv pass helper ===============
            def conv_pass(widx, wt, stats, prep, u_pool):
                """One binary conv over all images.

                prep(n, s) emits the sign-write of image n into s tile.
                stats col j = (k*2 + c)*28 + n*7 + b.
                Returns list of u tiles [128, 2, H, W] fp16 (prelu in y-units).
                """
                cm_col = CV_C1M if widx == 0 else CV_C2M
                u_tiles = []
                prep(0, s_tiles[0])
                for n in range(BL):
                    s = s_tiles[n % 2]
                    # emit the next image's sign-prep before this image's
                    # band work so ACT runs it early (software pipelining)
                    if n + 1 < BL:
                        prep(n + 1, s_tiles[(n + 1) % 2])
                    ut = u_pool.tile([128, 2, H, W], F16, tag=f"u{widx}", name=f"u{widx}_{n}")
                    u_tiles.append(ut)
                    for cc in range(2):
                        for b in range(NBAND):
                            pt = psum_pool.tile(
                                [128, NFREE], F32, tag="pt",
                                name=f"pt{widx}_{n}_{cc}_{b}",
                            )
                            for o, (dh, dw) in enumerate(OFFS):
                                start = (b * BAND + dh) * PW + dw
                                nc.tensor.matmul(
                                    pt[:],
                                    wt[:, :, (o * 2 + cc) * 128:(o * 2 + cc + 1) * 128],
                                    s[:, :, start:start + NFREE],
                                    start=(o == 0), stop=(o == 8),
                                    perf_mode=DR,
                                )
                            pv = pt[:].rearrange("p (r w) -> p r w", w=PW)[:, :, 0:W]
                            rsc = scrpool.tile([128, BAND, W], F16, tag="rsc", bufs=3)
                            sq = scrpool.tile([128, BAND, W], F32, tag="sq", bufs=1)
                            j0 = (0 * 2 + cc) * 28 + n * NBAND + b
                            j1 = (1 * 2 + cc) * 28 + n * NBAND + b
                            us = ut[:, cc, b * BAND:(b + 1) * BAND, :]
                            # rsc = relu(-y) = max(-y, 0)
                            nc.vector.scalar_tensor_tensor(
                                rsc[:], pv, -1.0, z16[:, 0:BAND, :],
                                op0=OP.mult, op1=OP.max,
                            )
                            # u = (1-a)*rsc + y, accum -> sum(u)
                            nc.vector.scalar_tensor_tensor(
                                us, rsc[:], cvec[:, cc, cm_col:cm_col + 1], pv,
                                op0=OP.mult, op1=OP.add,
                                accum_out=stats[:, j0:j0 + 1],
                            )
                            # sum(u^2) via ACT Square accum
                            nc.scalar.activation(
                                sq[:], us, AF.Square, bias=0.0, scale=1.0,
                                accum_out=stats[:, j1:j1 + 1],
                            )
                return u_tiles

            # =============== conv1 ===============
            def prep1(n, s):
                xt = x1s[n]
                for g in range(2):
                    nc.scalar.activation(
                        s_plane(s, g)[:, 1:57, 1:57],
                        xt[:, g, :].rearrange("p (h w) -> p h w", w=W),
                        AF.Sign, bias=negt1[:, g:g + 1], scale=1.0,
                    )

            u1 = conv_pass(0, w1t, st2, prep1, u1_pool)
            es_xs.close()  # x tiles for ph0/ph1 fully consumed

            # reduce st2 -> r2, AllReduce -> g2
            r2 = cpool.tile([128, 4], F32, tag="r2")
            nc.vector.reduce_sum(
                r2[:].rearrange("p (a b) -> p a b", b=1),
                st2[:].rearrange("p (kc t) -> p kc t", t=28),
                axis=mybir.AxisListType.X,
            )
            ar2_i = dram_pool.tile([128, 4], F32, tag="ar2_i")
            ar2_o = dram_pool.tile([NCORES, 128, 4], F32, tag="ar2_o", addr_space="Shared")
            nc.sync.dma_start(ar2_i[:], r2[:])
            nc.gpsimd.collective_compute(
                "AllGather", OP.bypass, replica_groups=[list(range(NCORES))],
                ins=[ar2_i[:].opt()], outs=[ar2_o[:].opt()],
            )
            gth2 = cpool.tile([128, 4, NCORES], F32, tag="gth2")
            nc.sync.dma_start(gth2[:], ar2_o[:].rearrange("r p k -> p k r"))
            nc.vector.reduce_sum(
                g2[:].rearrange("p (a b) -> p a b", b=1), gth2[:],
                axis=mybir.AxisListType.X,
            )

            # ---- BN2 threshold in u1 units ----
            g2v = g2[:].rearrange("p (k c) -> p k c", k=2)
            nc.vector.tensor_mul(tmp_a[:], g2v[:, 0], cvec[:, :, CV_SF1N])    # m2
            nc.vector.tensor_mul(tmp_b[:], g2v[:, 1], cvec[:, :, CV_SF1SQN])  # E[p1^2]
            nc.vector.scalar_tensor_tensor(
                tmp_c[:], tmp_a[:], -1.0, tmp_a[:], op0=OP.mult, op1=OP.mult,
            )
            nc.vector.tensor_add(tmp_c[:], tmp_c[:], tmp_b[:])                # v2
            nc.scalar.activation(tmp_b[:], tmp_c[:], AF.Sqrt, bias=cvec[:, 0, CV_EPS:CV_EPS + 1], scale=1.0)  # std2
            nc.vector.tensor_mul(tmp_c[:], tmp_b[:], cvec[:, :, CV_B2])       # B2*std2
            nc.vector.tensor_sub(tmp_a[:], tmp_a[:], tmp_c[:])                # t2 = m2 - B2*std2
            nc.vector.tensor_mul(tmp_a[:], tmp_a[:], cvec[:, :, CV_ISF1])     # theta (u units)
            nc.vector.tensor_scalar_mul(negth2[:], tmp_a[:], -1.0)

            # =============== conv2 ===============
            def prep2(n, s):
                for g in range(2):
                    nc.scalar.activation(
                        s_plane(s, g)[:, 1:57, 1:57],
                        u1[n][:, g, :, :],
                        AF.Sign, bias=negth2[:, g:g + 1], scale=1.0,
                    )

            u2 = conv_pass(1, w2t, st3, prep2, u2_pool)

            # u1 fully consumed by prep2; release its pool so the queue
            # allocator can reuse the region for phase-3 tiles
            es_u1.close()
            px3 = es_u2.enter_context(tc.tile_pool(name="px3", bufs=3))

            # prefetch x for phase 3 as fp16 (gpsimd DMA casts in flight;
            # residual-add precision ~5e-4 rel, well under tolerance)
            x3 = []
            for n in range(BL):
                xt = px3.tile([128, 2, HW], F16, tag="x3", name=f"x3_{n}")
                nc.gpsimd.dma_start(xt[:], x_view(n))
                x3.append(xt)

            # reduce st3 -> r3, AllReduce -> g3t
            r3 = cpool.tile([128, 4], F32, tag="r3")
            nc.vector.reduce_sum(
                r3[:].rearrange("p (a b) -> p a b", b=1),
                st3[:].rearrange("p (kc t) -> p kc t", t=28),
                axis=mybir.AxisListType.X,
            )
            ar3_i = dram_pool.tile([128, 4], F32, tag="ar3_i")
            ar3_o = dram_pool.tile([NCORES, 128, 4], F32, tag="ar3_o", addr_space="Shared")
            nc.sync.dma_start(ar3_i[:], r3[:])
            nc.gpsimd.collective_compute(
                "AllGather", OP.bypass, replica_groups=[list(range(NCORES))],
                ins=[ar3_i[:].opt()], outs=[ar3_o[:].opt()],
            )
            gth3 = cpool.tile([128, 4, NCORES], F32, tag="gth3")
            nc.sync.dma_start(gth3[:], ar3_o[:].rearrange("r p k -> p k r"))
            nc.vector.reduce_sum(
                g3t[:].rearrange("p (a b) -> p a b", b=1), gth3[:],
                axis=mybir.AxisListType.X,
            )

            # ---- BN3 affine: K = g3*sf2*rstd3, D = b3 - m3*g3*rstd3 ----
            g3v = g3t[:].rearrange("p (k c) -> p k c", k=2)
            nc.vector.tensor_mul(tmp_a[:], g3v[:, 0], cvec[:, :, CV_SF2N])    # m3
            nc.vector.tensor_mul(tmp_b[:], g3v[:, 1], cvec[:, :, CV_SF2SQN])  # E[p2^2]
            nc.vector.scalar_tensor_tensor(
                tmp_c[:], tmp_a[:], -1.0, tmp_a[:], op0=OP.mult, op1=OP.mult,
            )
            nc.vector.tensor_add(tmp_c[:], tmp_c[:], tmp_b[:])                # v3
            nc.scalar.activation(tmp_b[:], tmp_c[:], AF.Sqrt, bias=cvec[:, 0, CV_EPS:CV_EPS + 1], scale=1.0)  # std3
            nc.vector.reciprocal(tmp_c[:], tmp_b[:])                          # rstd3
            nc.vector.tensor_mul(kvec[:], tmp_c[:], cvec[:, :, CV_G3SF2])     # K
            nc.vector.tensor_mul(tmp_a[:], tmp_a[:], cvec[:, :, CV_G3])       # m3*g3
            nc.vector.tensor_mul(tmp_a[:], tmp_a[:], tmp_c[:])                # m3*g3*rstd3
            nc.vector.tensor_sub(dvec[:], cvec[:, :, CV_B3V], tmp_a[:])       # D

            # =============== phase 3: out = prelu(K*u2 + D + x, a3) ===============
            with tc.tile_pool(name="ph3", bufs=3) as p3pool:
                for n in range(BL):
                    for g in range(2):
                        # fp16 intermediates: w = K*u2 + D + x, out = prelu(w, a3)
                        th = p3pool.tile([128, H, W], F16, tag="th", name=f"th_{n}_{g}")
                        ot = p3pool.tile([128, H, W], F32, tag="ot", name=f"ot_{n}_{g}")
                        nc.scalar.activation(
                            th[:], u2[n][:, g, :, :], AF.Identity,
                            bias=dvec[:, g:g + 1], scale=kvec[:, g:g + 1],
                        )
                        nc.vector.scalar_tensor_tensor(
                            th[:], th[:], 0.0,
                            x3[n][:, g, :].rearrange("p (h w) -> p h w", w=W),
                            op0=OP.add, op1=OP.add,
                        )
                        nc.vector.scalar_tensor_tensor(
                            ot[:], th[:], cvec[:, g, CV_A3:CV_A3 + 1], th[:],
                            op0=OP.mult, op1=OP.max,
                        )
                        nc.sync.dma_start(out_view(n, g), ot[:])

    nc.compile()
    return nc


def _host_prep(inputs):
    x = np.ascontiguousarray(np.asarray(inputs["x"], dtype=np.float32))
    w1 = np.asarray(inputs["w1"], dtype=np.float32)
    w2 = np.asarray(inputs["w2"], dtype=np.float32)

    def wprep(w):
        ws = np.sign(w).astype(np.float32)  # [co, ci, kh, kw]
        sf = np.abs(w).mean(axis=(1, 2, 3)).astype(np.float32)  # [256]
        arr = np.empty((128, 2, 18, 128), dtype=np.float32)
        for o, (dh, dw) in enumerate(OFFS):
            for cc in range(2):
                t = ws[cc * 128:(cc + 1) * 128, :, dh, dw]  # [m, ci]
                # arr[p, g, blk, m] = t[m, g*128 + p]
                arr[:, :, o * 2 + cc, :] = t.T.reshape(2, 128, 128).transpose(1, 0, 2)
        return arr.reshape(128, 2, 18 * 128).astype(F8NP), sf

    w1s, sf1 = wprep(w1)
    w2s, sf2 = wprep(w2)

    def vec(v):
        return np.asarray(v, dtype=np.float32).reshape(2, 128).T  # [p, g]

    g1v, b1v = inputs["g1"], inputs["b1"]
    g2v, b2v = inputs["g2"], inputs["b2"]
    g3v, b3v = inputs["g3"], inputs["b3"]
    a1, a2, a3 = inputs["a1"], inputs["a2"], inputs["a3"]

    cvec = np.zeros((128, 2, CV_NCOLS), dtype=np.float32)
    cvec[:, :, CV_B1] = vec(np.asarray(b1v) / np.asarray(g1v))
    cvec[:, :, CV_C1M] = vec(1.0 - np.asarray(a1))
    cvec[:, :, CV_SF1N] = vec(sf1 / NTOT)
    cvec[:, :, CV_SF1SQN] = vec(sf1 * sf1 / NTOT)
    cvec[:, :, CV_B2] = vec(np.asarray(b2v) / np.asarray(g2v))
    cvec[:, :, CV_ISF1] = vec(1.0 / sf1)
    cvec[:, :, CV_IA1] = vec(1.0 / np.asarray(a1))
    cvec[:, :, CV_C2M] = vec(1.0 - np.asarray(a2))
    cvec[:, :, CV_SF2N] = vec(sf2 / NTOT)
    cvec[:, :, CV_SF2SQN] = vec(sf2 * sf2 / NTOT)
    cvec[:, :, CV_B3] = vec(np.asarray(b3v) / np.asarray(g3v))
    cvec[:, :, CV_G3SF2] = vec(np.asarray(g3v) * sf2)
    cvec[:, :, CV_G3] = vec(np.asarray(g3v))
    cvec[:, :, CV_B3V] = vec(np.asarray(b3v))
    cvec[:, :, CV_A3] = vec(np.asarray(a3))
    cvec[:, :, CV_EPS] = EPS

    return x, w1s, w2s, cvec


def run(inputs, trace=False):
    x, w1s, w2s, cvec = _host_prep(inputs)
    if "nc" not in _CACHE:
        _CACHE["nc"] = _build()
    nc = _CACHE["nc"]
    in_maps = [
        {"x": x[i * BL:(i + 1) * BL], "w1s": w1s, "w2s": w2s, "cvec": cvec}
        for i in range(NCORES)
    ]
    res = run_bass_kernel_spmd(nc, in_maps, list(range(NCORES)), trace=trace)
    out = np.concatenate([res.results[i]["out"] for i in range(NCORES)], axis=0)
    return out.astype(np.float32), res


def kernel(**inputs):
    out, _ = run(inputs, trace=False)
    return out


if __name__ == "__main__":
    # build-only check
    _build()
    print("BUILD OK")

